# revision 8
# baseline (speedup 1.0000x reference)
"""Trainium2 8-core tensor-parallel attention kernel (Bass/Tile).

Sharding: heads tensor-parallel across 8 cores (2 heads/core).
wq/wk/wv column-sharded by head, wo row-sharded; x replicated.
Chunked ReduceScatter (bf16) after the output projection; the host
concatenates the per-core row shards into the full output.

Self-contained: hardcodes B=2, S=2048, DIM=2048, NH=16, HD=128.
"""
import math

import numpy as np

B, S_FULL, DIM, NH = 2, 2048, 2048, 16
HD = 128
N_CORES = 8
HPC = NH // N_CORES          # heads per core
OC = HPC * HD                # output channels per core (256)
DT = DIM // 128              # d-tiles (16)
SC_W = 512                   # schunk width (cols of flattened seq)
RS_ROWS = 512                # rows per ReduceScatter chunk

_CACHE = {}


def _build(S):
    """Build the 8-core SPMD Bass graph for sequence length S (B=2 fixed)."""
    import concourse.bass as bass
    import concourse.mybir as mybir
    import concourse.tile as tile
    from concourse import bacc

    fp32 = mybir.dt.float32
    bf16 = mybir.dt.bfloat16
    Exp = mybir.ActivationFunctionType.Exp
    Copy = mybir.ActivationFunctionType.Copy
    AX = mybir.AxisListType.X
    AXC = mybir.AxisListType.C
    ADD = mybir.AluOpType.add
    MUL = mybir.AluOpType.mult

    FLAT = B * S                 # flattened rows
    NSC = FLAT // SC_W           # schunks in phase 1
    NQT = S // 128               # q-tiles per batch
    NQG = NQT // 4               # q-groups of 4 tiles per batch
    NCH = FLAT // RS_ROWS        # ReduceScatter chunks
    SCALE = 1.0 / math.sqrt(HD)
    rg = [list(range(N_CORES))]

    nc = bacc.Bacc("TRN2", target_bir_lowering=False, debug=False,
                   num_devices=N_CORES)

    # ---- external parameters ----
    xt_d = nc.declare_dram_parameter("xt", [DIM, FLAT], bf16, isOutput=False)
    wqt_d = nc.declare_dram_parameter("wqt", [DIM, OC], bf16, isOutput=False)
    wkt_d = nc.declare_dram_parameter("wkt", [DIM, OC], bf16, isOutput=False)
    wvt_d = nc.declare_dram_parameter("wvt", [DIM, OC], bf16, isOutput=False)
    wot_d = nc.declare_dram_parameter("wot", [OC, DIM], bf16, isOutput=False)
    cos_d = nc.declare_dram_parameter("cos_t", [HD, S], bf16, isOutput=False)
    sin_d = nc.declare_dram_parameter("sin_t", [HD, S], bf16, isOutput=False)
    mdg_d = nc.declare_dram_parameter("mask_diag", [NQT, 128, 128], fp32, isOutput=False)
    idn_d = nc.declare_dram_parameter("ident_bf", [128, 128], bf16, isOutput=False)
    rot_d = nc.declare_dram_parameter("rotp", [128, 128], bf16, isOutput=False)
    out_d = nc.declare_dram_parameter("out", [NCH, RS_ROWS // N_CORES, DIM], bf16,
                                      isOutput=True)

    # ---- internal DRAM ----
    qT_d = nc.dram_tensor("qT_dram", [HPC, 128, FLAT], bf16)
    kT_d = nc.dram_tensor("kT_dram", [HPC, 128, FLAT], bf16)
    vT_d = nc.dram_tensor("vT_dram", [HPC, 128, FLAT], bf16)
    par_d = nc.dram_tensor("partial_dram", [FLAT, DIM], bf16)
    rs_d = nc.dram_tensor("rs_out", [NCH, RS_ROWS // N_CORES, DIM], bf16)

    with tile.TileContext(nc) as tc:
        with tc.tile_pool(name="consts", bufs=1) as cpool:
            wot_sb = cpool.tile([128, HPC, DIM], bf16)
            nc.sync.dma_start(wot_sb[:], wot_d[:].rearrange("(h p) e -> p h e", p=128))
            cos_sb = cpool.tile([HD, S], bf16)
            nc.sync.dma_start(cos_sb[:], cos_d[:])
            sin_sb = cpool.tile([HD, S], bf16)
            nc.sync.dma_start(sin_sb[:], sin_d[:])
            mdg_sb = cpool.tile([128, NQT, 128], fp32)
            nc.sync.dma_start(mdg_sb[:], mdg_d[:].rearrange("t p k -> p t k"))
            idn_sb = cpool.tile([128, 128], bf16)
            nc.sync.dma_start(idn_sb[:], idn_d[:])
            rot_sb = cpool.tile([128, 128], bf16)
            nc.sync.dma_start(rot_sb[:], rot_d[:])

            # ================= phase 1: QKV projections (transposed) ======
            with (
                tc.tile_pool(name="wqkv", bufs=1) as wpool,
                tc.tile_pool(name="xT", bufs=6) as xpool,
                tc.tile_pool(name="p1sb", bufs=3) as spool,
                tc.tile_pool(name="p1tmp", bufs=2) as tpool,
                tc.tile_pool(name="qkvps", bufs=6, space="PSUM") as qkvps,
                tc.tile_pool(name="rotps", bufs=2, space="PSUM") as rotps,
            ):
                w_sb = {}
                for nm, d in (("q", wqt_d), ("k", wkt_d), ("v", wvt_d)):
                    w_sb[nm] = wpool.tile([128, DT, OC], bf16, tag=f"w{nm}", name=f"w{nm}")
                    nc.sync.dma_start(
                        w_sb[nm][:], d[:].rearrange("(t p) o -> p t o", p=128))

                for sc in range(NSC):
                    s0 = (sc * SC_W) % S  # position offset within batch
                    # transposed x chunk (x shipped pre-transposed)
                    xts = []
                    for dt in range(DT):
                        xt = xpool.tile([128, SC_W], bf16, tag="xt", name=f"xt{dt}")
                        nc.sync.dma_start(
                            xt[:], xt_d[dt * 128:(dt + 1) * 128,
                                        sc * SC_W:(sc + 1) * SC_W])
                        xts.append(xt)
                    # projections: psum[t,h] += w[t][dt,h].T @ xT[dt]
                    ps = {}
                    for t in ("q", "k", "v"):
                        for h in range(HPC):
                            ps[(t, h)] = qkvps.tile([128, SC_W], fp32, tag="qkv", name=f"ps_{t}{h}")
                    for dt in range(DT):
                        for t in ("q", "k", "v"):
                            for h in range(HPC):
                                nc.tensor.matmul(
                                    ps[(t, h)][:],
                                    w_sb[t][:, dt, h * HD:(h + 1) * HD],
                                    xts[dt][:],
                                    start=(dt == 0), stop=(dt == DT - 1))
                    # RoPE for q, k; plain copy-out for v
                    for h in range(HPC):
                        for t, dram, scale in (("q", qT_d, SCALE), ("k", kT_d, 1.0)):
                            til = spool.tile([128, SC_W], bf16, tag="til")
                            nc.scalar.activation(til[:], ps[(t, h)][:], Copy,
                                                 scale=scale)
                            rp = rotps.tile([128, SC_W], fp32, tag="rot")
                            nc.tensor.matmul(rp[:], rot_sb[:], til[:],
                                             start=True, stop=True)
                            t1 = tpool.tile([128, SC_W], bf16, tag="t1")
                            nc.vector.tensor_mul(t1[:], til[:],
                                                 cos_sb[:, s0:s0 + SC_W])
                            hat = spool.tile([128, SC_W], bf16, tag="hat")
                            nc.vector.tensor_mul(hat[:], rp[:],
                                                 sin_sb[:, s0:s0 + SC_W])
                            nc.vector.tensor_add(hat[:], hat[:], t1[:])
                            nc.sync.dma_start(
                                dram[h, :, sc * SC_W:(sc + 1) * SC_W], hat[:])
                        vb = spool.tile([128, SC_W], bf16, tag="vb")
                        nc.scalar.copy(vb[:], ps[("v", h)][:])
                        nc.sync.dma_start(
                            vT_d[h, :, sc * SC_W:(sc + 1) * SC_W], vb[:])

            # ================= phase 2: attention + O-proj + RS ===========
            # scoresT route: scoresT[k,q] = kT_tile.T @ qT (born transposed),
            # exp lands directly in the PV staging buffer; column sums on
            # GpSimd; normalization deferred to the output tile.
            with (
                tc.tile_pool(name="qk_sb", bufs=2) as qkpool,
                tc.tile_pool(name="vbf", bufs=2) as vpool,
                tc.tile_pool(name="probsT", bufs=2) as ptpool,
                tc.tile_pool(name="outT", bufs=2) as opool,
                tc.tile_pool(name="small", bufs=4) as smpool,
                tc.tile_pool(name="partial", bufs=3) as papool,
                tc.tile_pool(name="scps", bufs=3, space="PSUM") as scps,
                tc.tile_pool(name="ops", bufs=2, space="PSUM") as ops,
                tc.tile_pool(name="pps", bufs=3, space="PSUM") as pps,
            ):
                for b in range(B):
                    oT = {}
                    for h in range(HPC):
                        qT = qkpool.tile([128, S], bf16, tag="qT")
                        nc.sync.dma_start(qT[:], qT_d[h, :, b * S:(b + 1) * S])
                        kT = qkpool.tile([128, S], bf16, tag="kT")
                        nc.sync.dma_start(kT[:], kT_d[h, :, b * S:(b + 1) * S])
                        vbf = vpool.tile([128, NQT, HD], bf16, tag="v")
                        for kt in range(NQT):
                            nc.sync.dma_start(
                                vbf[:, kt, :],
                                vT_d[h, :, b * S + kt * 128: b * S + (kt + 1) * 128],
                                transpose=True)
                        oT[h] = opool.tile([128, S], bf16, tag="oT", name=f"oT{h}")

                        for qg in range(NQG):
                            kmax = qg * 4 + 3
                            pT = ptpool.tile([128, NQT, 512], bf16, tag="pT")
                            po = ops.tile([128, 512], fp32, tag="po")
                            acc_row = smpool.tile([1, 512], fp32, tag="acc")
                            nc.gpsimd.memset(acc_row[:], 0.0)
                            for kt in range(kmax + 1):
                                qlo = max(0, kt - qg * 4) * 128
                                n = 512 - qlo
                                sp = scps.tile([128, 512], fp32, tag="sc")
                                nc.tensor.matmul(
                                    sp[:, :n],
                                    kT[:, kt * 128:(kt + 1) * 128],
                                    qT[:, qg * 512 + qlo: (qg + 1) * 512],
                                    start=True, stop=True)
                                if kt >= qg * 4:  # diag tile sits at local cols 0:128
                                    nc.vector.tensor_add(
                                        sp[:, 0:128], sp[:, 0:128], mdg_sb[:, kt, :])
                                nc.scalar.activation(
                                    pT[:, kt, qlo:512], sp[:, :n], Exp)
                                trow = smpool.tile([1, 512], fp32, tag="trow")
                                nc.gpsimd.reduce_sum(
                                    trow[:, :n], pT[:, kt, qlo:512], axis=AXC)
                                nc.gpsimd.tensor_add(
                                    acc_row[:, qlo:512], acc_row[:, qlo:512],
                                    trow[:, :n])
                                if kt >= 1:
                                    klast = kt - 1
                                    ql2 = max(0, klast - qg * 4) * 128
                                    nc.tensor.matmul(
                                        po[:, ql2:512], vbf[:, klast, :],
                                        pT[:, klast, ql2:512],
                                        start=(klast == 0), stop=False)
                            nc.tensor.matmul(
                                po[:, 384:512], vbf[:, kmax, :],
                                pT[:, kmax, 384:512], start=False, stop=True)
                            # normalization for this q-column group
                            rrow = smpool.tile([1, 512], fp32, tag="rrow")
                            nc.vector.reciprocal(rrow[:], acc_row[:])
                            rbc = smpool.tile([128, 512], fp32, tag="rbc")
                            nc.gpsimd.partition_broadcast(rbc[:], rrow[:])
                            nc.vector.tensor_mul(
                                oT[h][:, qg * 512:(qg + 1) * 512], po[:], rbc[:])

                    # ---- O-projection for batch b + chunked ReduceScatter ----
                    for st in range(NQT):
                        pp = [pps.tile([128, 512], fp32, tag="pp", name=f"pp{e}") for e in range(4)]
                        for h in range(HPC):
                            for ec in range(4):
                                nc.tensor.matmul(
                                    pp[ec][:],
                                    oT[h][:, st * 128:(st + 1) * 128],
                                    wot_sb[:, h, ec * 512:(ec + 1) * 512],
                                    start=(h == 0), stop=(h == HPC - 1))
                        par = papool.tile([128, DIM], bf16, tag="par")
                        for ec in range(4):
                            nc.scalar.copy(par[:, ec * 512:(ec + 1) * 512], pp[ec][:])
                        nc.sync.dma_start(
                            par_d[b * S + st * 128: b * S + (st + 1) * 128, :], par[:])
                        # ReduceScatter every RS_ROWS rows
                        if (st + 1) % (RS_ROWS // 128) == 0:
                            ch = (b * S + (st + 1) * 128) // RS_ROWS - 1
                            nc.gpsimd.collective_compute(
                                "ReduceScatter", ADD, replica_groups=rg,
                                ins=[par_d[ch * RS_ROWS:(ch + 1) * RS_ROWS, :]],
                                outs=[rs_d[ch]])
                            nc.sync.dma_start(out_d[ch], rs_d[ch])

    nc.compile()
    return nc


def _get_nc(S):
    if S not in _CACHE:
        _CACHE[S] = _build(S)
    return _CACHE[S]


def make_inputs(x, freqs_cis, mask, wq, wk, wv, wo):
    """Host-side sharding / layout prep. Returns in_maps for 8 cores."""
    S = x.shape[1]
    flat_xt = np.ascontiguousarray(np.asarray(x, np.float32).reshape(B * S, DIM).T)
    cos = np.asarray(freqs_cis[..., 0], np.float32)   # [S, HD/2]
    sin = np.asarray(freqs_cis[..., 1], np.float32)
    cos_t = np.ascontiguousarray(np.repeat(cos.T, 2, axis=0))  # [HD, S]
    sin_t = np.ascontiguousarray(np.repeat(sin.T, 2, axis=0))
    m = np.asarray(mask, np.float32)[0, 0]
    nqt = S // 128
    mask_diag = np.ascontiguousarray(
        np.stack([m[i * 128:(i + 1) * 128, i * 128:(i + 1) * 128].T
                  for i in range(nqt)]))
    import ml_dtypes
    bf = ml_dtypes.bfloat16
    flat_xt = flat_xt.astype(bf)
    cos_t = cos_t.astype(bf)
    sin_t = sin_t.astype(bf)
    ident_bf = np.eye(128, dtype=bf)
    P = np.zeros((128, 128), np.float32)
    for j in range(64):
        P[2 * j, 2 * j + 1] = -1.0
        P[2 * j + 1, 2 * j] = 1.0
    rotp = np.ascontiguousarray(P.T)

    in_maps = []
    for c in range(N_CORES):
        r = slice(c * OC, (c + 1) * OC)
        in_maps.append({
            "xt": flat_xt,
            "wqt": np.ascontiguousarray(np.asarray(wq, np.float32)[r, :].T).astype(bf),
            "wkt": np.ascontiguousarray(np.asarray(wk, np.float32)[r, :].T).astype(bf),
            "wvt": np.ascontiguousarray(np.asarray(wv, np.float32)[r, :].T).astype(bf),
            "wot": np.ascontiguousarray(np.asarray(wo, np.float32)[:, r].T).astype(bf),
            "cos_t": cos_t,
            "sin_t": sin_t,
            "mask_diag": mask_diag,
            "ident_bf": ident_bf,
            "rotp": rotp.astype(bf),
        })
    return in_maps


def assemble(results, S):
    """Concatenate per-core ReduceScatter shards into the full output."""
    nch = B * S // RS_ROWS
    shards = [np.asarray(results[c]["out"], dtype=np.float32)
                .reshape(nch, RS_ROWS // N_CORES, DIM)
              for c in range(N_CORES)]
    full = np.empty((nch, N_CORES, RS_ROWS // N_CORES, DIM), np.float32)
    for c in range(N_CORES):
        full[:, c] = shards[c]
    return full.reshape(B, S, DIM)


def kernel(x, start_pos, freqs_cis, mask, wq, wk, wv, wo):
    from concourse.bass_utils import run_bass_kernel_spmd
    S = x.shape[1]
    nc = _get_nc(S)
    in_maps = make_inputs(x, freqs_cis, mask, wq, wk, wv, wo)
    res = run_bass_kernel_spmd(nc, in_maps, core_ids=list(range(N_CORES)))
    return assemble(res.results, S)


# revision 9
# speedup vs baseline: 13.2284x; 13.2284x over previous
"""Trainium2 8-core tensor-parallel attention kernel (Bass/Tile).

Sharding: heads tensor-parallel across 8 cores (2 heads/core).
wq/wk/wv column-sharded by head, wo row-sharded; x replicated.
Chunked ReduceScatter (bf16) after the output projection; the host
concatenates the per-core row shards into the full output.

Self-contained: hardcodes B=2, S=2048, DIM=2048, NH=16, HD=128.
"""
import math

import numpy as np

B, S_FULL, DIM, NH = 2, 2048, 2048, 16
HD = 128
N_CORES = 8
HPC = NH // N_CORES          # heads per core
OC = HPC * HD                # output channels per core (256)
DT = DIM // 128              # d-tiles (16)
SC_W = 512                   # schunk width (cols of flattened seq)
RS_ROWS = 512                # rows per ReduceScatter chunk

_CACHE = {}


def _build(S):
    """Build the 8-core SPMD Bass graph for sequence length S (B=2 fixed)."""
    import concourse.bass as bass
    import concourse.mybir as mybir
    import concourse.tile as tile
    from concourse import bacc

    fp32 = mybir.dt.float32
    bf16 = mybir.dt.bfloat16
    Exp = mybir.ActivationFunctionType.Exp
    Copy = mybir.ActivationFunctionType.Copy
    AX = mybir.AxisListType.X
    AXC = mybir.AxisListType.C
    ADD = mybir.AluOpType.add
    MUL = mybir.AluOpType.mult

    FLAT = B * S                 # flattened rows
    NSC = FLAT // SC_W           # schunks in phase 1
    NQT = S // 128               # q-tiles per batch
    NQG = NQT // 4               # q-groups of 4 tiles per batch
    NCH = FLAT // RS_ROWS        # ReduceScatter chunks
    SCALE = 1.0 / math.sqrt(HD)
    rg = [list(range(N_CORES))]

    nc = bacc.Bacc("TRN2", target_bir_lowering=False, debug=False,
                   num_devices=N_CORES)

    # ---- external parameters ----
    xt_d = nc.declare_dram_parameter("xt", [DIM, FLAT], bf16, isOutput=False)
    wqt_d = nc.declare_dram_parameter("wqt", [DIM, OC], bf16, isOutput=False)
    wkt_d = nc.declare_dram_parameter("wkt", [DIM, OC], bf16, isOutput=False)
    wvt_d = nc.declare_dram_parameter("wvt", [DIM, OC], bf16, isOutput=False)
    wot_d = nc.declare_dram_parameter("wot", [OC, DIM], bf16, isOutput=False)
    cos_d = nc.declare_dram_parameter("cos_t", [HD, S], bf16, isOutput=False)
    sin_d = nc.declare_dram_parameter("sin_t", [HD, S], bf16, isOutput=False)
    mdg_d = nc.declare_dram_parameter("mask_diag", [NQT, 128, 128], fp32, isOutput=False)
    idn_d = nc.declare_dram_parameter("ident_bf", [128, 128], bf16, isOutput=False)
    rot_d = nc.declare_dram_parameter("rotp", [128, 128], bf16, isOutput=False)
    one_d = nc.declare_dram_parameter("ones_bf", [128, 1], bf16, isOutput=False)
    out_d = nc.declare_dram_parameter("out", [NCH, RS_ROWS // N_CORES, DIM], bf16,
                                      isOutput=True)

    # ---- internal DRAM ----
    qT_d = nc.dram_tensor("qT_dram", [HPC, 128, FLAT], bf16)
    kT_d = nc.dram_tensor("kT_dram", [HPC, 128, FLAT], bf16)
    vT_d = nc.dram_tensor("vT_dram", [HPC, 128, FLAT], bf16)
    par_d = nc.dram_tensor("partial_dram", [FLAT, DIM], bf16)
    rs_d = nc.dram_tensor("rs_out", [NCH, RS_ROWS // N_CORES, DIM], bf16)

    with tile.TileContext(nc) as tc:
        with tc.tile_pool(name="consts", bufs=1) as cpool:
            wot_sb = cpool.tile([128, HPC, DIM], bf16)
            nc.sync.dma_start(wot_sb[:], wot_d[:].rearrange("(h p) e -> p h e", p=128))
            cos_sb = cpool.tile([HD, S], bf16)
            nc.sync.dma_start(cos_sb[:], cos_d[:])
            sin_sb = cpool.tile([HD, S], bf16)
            nc.sync.dma_start(sin_sb[:], sin_d[:])
            mdg_sb = cpool.tile([128, NQT, 128], fp32)
            nc.sync.dma_start(mdg_sb[:], mdg_d[:].rearrange("t p k -> p t k"))
            idn_sb = cpool.tile([128, 128], bf16)
            nc.sync.dma_start(idn_sb[:], idn_d[:])
            rot_sb = cpool.tile([128, 128], bf16)
            nc.sync.dma_start(rot_sb[:], rot_d[:])
            one_sb = cpool.tile([128, 1], bf16)
            nc.sync.dma_start(one_sb[:], one_d[:])

            # ================= phase 1: QKV projections (transposed) ======
            with (
                tc.tile_pool(name="wqkv", bufs=1) as wpool,
                tc.tile_pool(name="xT", bufs=6) as xpool,
                tc.tile_pool(name="p1sb", bufs=3) as spool,
                tc.tile_pool(name="p1tmp", bufs=2) as tpool,
                tc.tile_pool(name="qkvps", bufs=6, space="PSUM") as qkvps,
                tc.tile_pool(name="rotps", bufs=2, space="PSUM") as rotps,
            ):
                w_sb = {}
                for nm, d in (("q", wqt_d), ("k", wkt_d), ("v", wvt_d)):
                    w_sb[nm] = wpool.tile([128, DT, OC], bf16, tag=f"w{nm}", name=f"w{nm}")
                    nc.sync.dma_start(
                        w_sb[nm][:], d[:].rearrange("(t p) o -> p t o", p=128))

                for sc in range(NSC):
                    s0 = (sc * SC_W) % S  # position offset within batch
                    # transposed x chunk (x shipped pre-transposed)
                    xts = []
                    for dt in range(DT):
                        xt = xpool.tile([128, SC_W], bf16, tag="xt", name=f"xt{dt}")
                        nc.sync.dma_start(
                            xt[:], xt_d[dt * 128:(dt + 1) * 128,
                                        sc * SC_W:(sc + 1) * SC_W])
                        xts.append(xt)
                    # projections: psum[t,h] += w[t][dt,h].T @ xT[dt]
                    ps = {}
                    for t in ("q", "k", "v"):
                        for h in range(HPC):
                            ps[(t, h)] = qkvps.tile([128, SC_W], fp32, tag="qkv", name=f"ps_{t}{h}")
                    for dt in range(DT):
                        for t in ("q", "k", "v"):
                            for h in range(HPC):
                                nc.tensor.matmul(
                                    ps[(t, h)][:],
                                    w_sb[t][:, dt, h * HD:(h + 1) * HD],
                                    xts[dt][:],
                                    start=(dt == 0), stop=(dt == DT - 1))
                    # RoPE for q, k; plain copy-out for v
                    for h in range(HPC):
                        for t, dram, scale in (("q", qT_d, SCALE), ("k", kT_d, 1.0)):
                            til = spool.tile([128, SC_W], bf16, tag="til")
                            nc.scalar.activation(til[:], ps[(t, h)][:], Copy,
                                                 scale=scale)
                            rp = rotps.tile([128, SC_W], fp32, tag="rot")
                            nc.tensor.matmul(rp[:], rot_sb[:], til[:],
                                             start=True, stop=True)
                            t1 = tpool.tile([128, SC_W], bf16, tag="t1")
                            nc.vector.tensor_mul(t1[:], til[:],
                                                 cos_sb[:, s0:s0 + SC_W])
                            hat = spool.tile([128, SC_W], bf16, tag="hat")
                            nc.vector.tensor_mul(hat[:], rp[:],
                                                 sin_sb[:, s0:s0 + SC_W])
                            nc.vector.tensor_add(hat[:], hat[:], t1[:])
                            nc.sync.dma_start(
                                dram[h, :, sc * SC_W:(sc + 1) * SC_W], hat[:])
                        vb = spool.tile([128, SC_W], bf16, tag="vb")
                        nc.scalar.copy(vb[:], ps[("v", h)][:])
                        nc.sync.dma_start(
                            vT_d[h, :, sc * SC_W:(sc + 1) * SC_W], vb[:])

            # ================= phase 2: attention + O-proj + RS ===========
            # scoresT route: scoresT[k,q] = kT_tile.T @ qT (born transposed),
            # exp lands directly in the PV staging buffer; column sums on
            # GpSimd; normalization deferred to the output tile.
            with (
                tc.tile_pool(name="qk_sb", bufs=2) as qkpool,
                tc.tile_pool(name="vbf", bufs=2) as vpool,
                tc.tile_pool(name="probsT", bufs=2) as ptpool,
                tc.tile_pool(name="outT", bufs=2) as opool,
                tc.tile_pool(name="small", bufs=4) as smpool,
                tc.tile_pool(name="partial", bufs=3) as papool,
                tc.tile_pool(name="scps", bufs=2, space="PSUM") as scps,
                tc.tile_pool(name="sups", bufs=1, space="PSUM") as sups,
                tc.tile_pool(name="ops", bufs=2, space="PSUM") as ops,
                tc.tile_pool(name="pps", bufs=3, space="PSUM") as pps,
            ):
                for b in range(B):
                    oT = {}
                    for h in range(HPC):
                        qT = qkpool.tile([128, S], bf16, tag="qT")
                        nc.sync.dma_start(qT[:], qT_d[h, :, b * S:(b + 1) * S])
                        kT = qkpool.tile([128, S], bf16, tag="kT")
                        nc.sync.dma_start(kT[:], kT_d[h, :, b * S:(b + 1) * S])
                        vbf = vpool.tile([128, NQT, HD], bf16, tag="v")
                        for kt in range(NQT):
                            nc.sync.dma_start(
                                vbf[:, kt, :],
                                vT_d[h, :, b * S + kt * 128: b * S + (kt + 1) * 128],
                                transpose=True)
                        oT[h] = opool.tile([128, S], bf16, tag="oT", name=f"oT{h}")

                        for qg in range(NQG):
                            kmax = qg * 4 + 3
                            pT = ptpool.tile([128, NQT, 512], bf16, tag="pT")
                            po = ops.tile([128, 512], fp32, tag="po")
                            sums_ps = sups.tile([1, 512], fp32, tag="sps")
                            for kt in range(kmax + 1):
                                qlo = max(0, kt - qg * 4) * 128
                                n = 512 - qlo
                                sp = scps.tile([128, 512], fp32, tag="sc")
                                nc.tensor.matmul(
                                    sp[:, :n],
                                    kT[:, kt * 128:(kt + 1) * 128],
                                    qT[:, qg * 512 + qlo: (qg + 1) * 512],
                                    start=True, stop=True)
                                if kt >= qg * 4:  # diag tile sits at local cols 0:128
                                    nc.vector.tensor_add(
                                        sp[:, 0:128], sp[:, 0:128], mdg_sb[:, kt, :])
                                nc.scalar.activation(
                                    pT[:, kt, qlo:512], sp[:, :n], Exp)
                                nc.tensor.matmul(
                                    sums_ps[:, qlo:512], one_sb[:],
                                    pT[:, kt, qlo:512],
                                    start=(kt == 0), stop=(kt == kmax))
                                if kt >= 1:
                                    klast = kt - 1
                                    ql2 = max(0, klast - qg * 4) * 128
                                    nc.tensor.matmul(
                                        po[:, ql2:512], vbf[:, klast, :],
                                        pT[:, klast, ql2:512],
                                        start=(klast == 0), stop=False)
                            nc.tensor.matmul(
                                po[:, 384:512], vbf[:, kmax, :],
                                pT[:, kmax, 384:512], start=False, stop=True)
                            # normalization for this q-column group
                            rrow = smpool.tile([1, 512], fp32, tag="rrow")
                            nc.vector.reciprocal(rrow[:], sums_ps[:])
                            rbc = smpool.tile([128, 512], fp32, tag="rbc")
                            nc.gpsimd.partition_broadcast(rbc[:], rrow[:])
                            nc.vector.tensor_mul(
                                oT[h][:, qg * 512:(qg + 1) * 512], po[:], rbc[:])

                    # ---- O-projection for batch b + chunked ReduceScatter ----
                    for st in range(NQT):
                        pp = [pps.tile([128, 512], fp32, tag="pp", name=f"pp{e}") for e in range(4)]
                        for h in range(HPC):
                            for ec in range(4):
                                nc.tensor.matmul(
                                    pp[ec][:],
                                    oT[h][:, st * 128:(st + 1) * 128],
                                    wot_sb[:, h, ec * 512:(ec + 1) * 512],
                                    start=(h == 0), stop=(h == HPC - 1))
                        par = papool.tile([128, DIM], bf16, tag="par")
                        for ec in range(4):
                            nc.scalar.copy(par[:, ec * 512:(ec + 1) * 512], pp[ec][:])
                        nc.sync.dma_start(
                            par_d[b * S + st * 128: b * S + (st + 1) * 128, :], par[:])
                        # ReduceScatter every RS_ROWS rows
                        if (st + 1) % (RS_ROWS // 128) == 0:
                            ch = (b * S + (st + 1) * 128) // RS_ROWS - 1
                            nc.gpsimd.collective_compute(
                                "ReduceScatter", ADD, replica_groups=rg,
                                ins=[par_d[ch * RS_ROWS:(ch + 1) * RS_ROWS, :]],
                                outs=[rs_d[ch]])
                            nc.sync.dma_start(out_d[ch], rs_d[ch])

    nc.compile()
    return nc


def _get_nc(S):
    if S not in _CACHE:
        _CACHE[S] = _build(S)
    return _CACHE[S]


def make_inputs(x, freqs_cis, mask, wq, wk, wv, wo):
    """Host-side sharding / layout prep. Returns in_maps for 8 cores."""
    S = x.shape[1]
    flat_xt = np.ascontiguousarray(np.asarray(x, np.float32).reshape(B * S, DIM).T)
    cos = np.asarray(freqs_cis[..., 0], np.float32)   # [S, HD/2]
    sin = np.asarray(freqs_cis[..., 1], np.float32)
    cos_t = np.ascontiguousarray(np.repeat(cos.T, 2, axis=0))  # [HD, S]
    sin_t = np.ascontiguousarray(np.repeat(sin.T, 2, axis=0))
    m = np.asarray(mask, np.float32)[0, 0]
    nqt = S // 128
    mask_diag = np.ascontiguousarray(
        np.stack([m[i * 128:(i + 1) * 128, i * 128:(i + 1) * 128].T
                  for i in range(nqt)]))
    import ml_dtypes
    bf = ml_dtypes.bfloat16
    flat_xt = flat_xt.astype(bf)
    cos_t = cos_t.astype(bf)
    sin_t = sin_t.astype(bf)
    ident_bf = np.eye(128, dtype=bf)
    P = np.zeros((128, 128), np.float32)
    for j in range(64):
        P[2 * j, 2 * j + 1] = -1.0
        P[2 * j + 1, 2 * j] = 1.0
    rotp = np.ascontiguousarray(P.T)

    in_maps = []
    for c in range(N_CORES):
        r = slice(c * OC, (c + 1) * OC)
        in_maps.append({
            "xt": flat_xt,
            "wqt": np.ascontiguousarray(np.asarray(wq, np.float32)[r, :].T).astype(bf),
            "wkt": np.ascontiguousarray(np.asarray(wk, np.float32)[r, :].T).astype(bf),
            "wvt": np.ascontiguousarray(np.asarray(wv, np.float32)[r, :].T).astype(bf),
            "wot": np.ascontiguousarray(np.asarray(wo, np.float32)[:, r].T).astype(bf),
            "cos_t": cos_t,
            "sin_t": sin_t,
            "mask_diag": mask_diag,
            "ident_bf": ident_bf,
            "rotp": rotp.astype(bf),
            "ones_bf": np.ones((128, 1), dtype=bf),
        })
    return in_maps


def assemble(results, S):
    """Concatenate per-core ReduceScatter shards into the full output."""
    nch = B * S // RS_ROWS
    shards = [np.asarray(results[c]["out"], dtype=np.float32)
                .reshape(nch, RS_ROWS // N_CORES, DIM)
              for c in range(N_CORES)]
    full = np.empty((nch, N_CORES, RS_ROWS // N_CORES, DIM), np.float32)
    for c in range(N_CORES):
        full[:, c] = shards[c]
    return full.reshape(B, S, DIM)


def kernel(x, start_pos, freqs_cis, mask, wq, wk, wv, wo):
    from concourse.bass_utils import run_bass_kernel_spmd
    S = x.shape[1]
    nc = _get_nc(S)
    in_maps = make_inputs(x, freqs_cis, mask, wq, wk, wv, wo)
    res = run_bass_kernel_spmd(nc, in_maps, core_ids=list(range(N_CORES)))
    return assemble(res.results, S)


# revision 11
# speedup vs baseline: 14.2100x; 1.0742x over previous
"""Trainium2 8-core tensor-parallel attention kernel (Bass/Tile).

Sharding: heads tensor-parallel across 8 cores (2 heads/core).
wq/wk/wv column-sharded by head, wo row-sharded; x replicated.
Chunked ReduceScatter (bf16) after the output projection; the host
concatenates the per-core row shards into the full output.

Self-contained: hardcodes B=2, S=2048, DIM=2048, NH=16, HD=128.
"""
import math

import numpy as np

B, S_FULL, DIM, NH = 2, 2048, 2048, 16
HD = 128
N_CORES = 8
HPC = NH // N_CORES          # heads per core
OC = HPC * HD                # output channels per core (256)
DT = DIM // 128              # d-tiles (16)
SC_W = 512                   # schunk width (cols of flattened seq)
RS_ROWS = 512                # rows per ReduceScatter chunk

_CACHE = {}


def _build(S):
    """Build the 8-core SPMD Bass graph for sequence length S (B=2 fixed)."""
    import concourse.bass as bass
    import concourse.mybir as mybir
    import concourse.tile as tile
    from concourse import bacc

    fp32 = mybir.dt.float32
    bf16 = mybir.dt.bfloat16
    Exp = mybir.ActivationFunctionType.Exp
    Copy = mybir.ActivationFunctionType.Copy
    AX = mybir.AxisListType.X
    AXC = mybir.AxisListType.C
    ADD = mybir.AluOpType.add
    MUL = mybir.AluOpType.mult

    FLAT = B * S                 # flattened rows
    NSC = FLAT // SC_W           # schunks in phase 1
    NQT = S // 128               # q-tiles per batch
    NQG = NQT // 4               # q-groups of 4 tiles per batch
    NCH = FLAT // RS_ROWS        # ReduceScatter chunks
    SCALE = 1.0 / math.sqrt(HD)
    rg = [list(range(N_CORES))]

    nc = bacc.Bacc("TRN2", target_bir_lowering=False, debug=False,
                   num_devices=N_CORES)

    # ---- external parameters ----
    xt_d = nc.declare_dram_parameter("xt", [DIM, FLAT], bf16, isOutput=False)
    wqt_d = nc.declare_dram_parameter("wqt", [DIM, OC], bf16, isOutput=False)
    wkt_d = nc.declare_dram_parameter("wkt", [DIM, OC], bf16, isOutput=False)
    wvt_d = nc.declare_dram_parameter("wvt", [DIM, OC], bf16, isOutput=False)
    wot_d = nc.declare_dram_parameter("wot", [OC, DIM], bf16, isOutput=False)
    cos_d = nc.declare_dram_parameter("cos_t", [HD, S], bf16, isOutput=False)
    sin_d = nc.declare_dram_parameter("sin_t", [HD, S], bf16, isOutput=False)
    mdg_d = nc.declare_dram_parameter("mask_diag", [NQT, 128, 128], fp32, isOutput=False)
    idn_d = nc.declare_dram_parameter("ident_bf", [128, 128], bf16, isOutput=False)
    rot_d = nc.declare_dram_parameter("rotp", [128, 128], bf16, isOutput=False)
    one_d = nc.declare_dram_parameter("ones_bf", [128, 1], bf16, isOutput=False)
    out_d = nc.declare_dram_parameter("out", [NCH, RS_ROWS // N_CORES, DIM], bf16,
                                      isOutput=True)

    # ---- internal DRAM ----
    qT_d = nc.dram_tensor("qT_dram", [HPC, 128, FLAT], bf16)
    kT_d = nc.dram_tensor("kT_dram", [HPC, 128, FLAT], bf16)
    vT_d = nc.dram_tensor("vT_dram", [HPC, 128, FLAT], bf16)
    par_d = nc.dram_tensor("partial_dram", [FLAT, DIM], bf16)
    rs_d = nc.dram_tensor("rs_out", [NCH, RS_ROWS // N_CORES, DIM], bf16)

    with tile.TileContext(nc) as tc:
        with tc.tile_pool(name="consts", bufs=1) as cpool:
            wot_sb = cpool.tile([128, HPC, DIM], bf16)
            nc.sync.dma_start(wot_sb[:], wot_d[:].rearrange("(h p) e -> p h e", p=128))
            cos_sb = cpool.tile([HD, S], bf16)
            nc.sync.dma_start(cos_sb[:], cos_d[:])
            sin_sb = cpool.tile([HD, S], bf16)
            nc.sync.dma_start(sin_sb[:], sin_d[:])
            mdg_sb = cpool.tile([128, NQT, 128], fp32)
            nc.sync.dma_start(mdg_sb[:], mdg_d[:].rearrange("t p k -> p t k"))
            idn_sb = cpool.tile([128, 128], bf16)
            nc.sync.dma_start(idn_sb[:], idn_d[:])
            rot_sb = cpool.tile([128, 128], bf16)
            nc.sync.dma_start(rot_sb[:], rot_d[:])
            one_sb = cpool.tile([128, 1], bf16)
            nc.sync.dma_start(one_sb[:], one_d[:])

            # ================= phase 1: QKV projections (transposed) ======
            with (
                tc.tile_pool(name="wqkv", bufs=1) as wpool,
                tc.tile_pool(name="xT", bufs=6) as xpool,
                tc.tile_pool(name="p1sb", bufs=3) as spool,
                tc.tile_pool(name="p1tmp", bufs=2) as tpool,
                tc.tile_pool(name="qkvps", bufs=6, space="PSUM") as qkvps,
                tc.tile_pool(name="rotps", bufs=2, space="PSUM") as rotps,
            ):
                w_sb = {}
                for nm, d in (("q", wqt_d), ("k", wkt_d), ("v", wvt_d)):
                    w_sb[nm] = wpool.tile([128, DT, OC], bf16, tag=f"w{nm}", name=f"w{nm}")
                    nc.sync.dma_start(
                        w_sb[nm][:], d[:].rearrange("(t p) o -> p t o", p=128))

                for sc in range(NSC):
                    s0 = (sc * SC_W) % S  # position offset within batch
                    # transposed x chunk (x shipped pre-transposed)
                    xts = []
                    for dt in range(DT):
                        xt = xpool.tile([128, SC_W], bf16, tag="xt", name=f"xt{dt}")
                        nc.sync.dma_start(
                            xt[:], xt_d[dt * 128:(dt + 1) * 128,
                                        sc * SC_W:(sc + 1) * SC_W])
                        xts.append(xt)
                    # projections: psum[t,h] += w[t][dt,h].T @ xT[dt]
                    ps = {}
                    for t in ("q", "k", "v"):
                        for h in range(HPC):
                            ps[(t, h)] = qkvps.tile([128, SC_W], fp32, tag="qkv", name=f"ps_{t}{h}")
                    for dt in range(DT):
                        for t in ("q", "k", "v"):
                            for h in range(HPC):
                                nc.tensor.matmul(
                                    ps[(t, h)][:],
                                    w_sb[t][:, dt, h * HD:(h + 1) * HD],
                                    xts[dt][:],
                                    start=(dt == 0), stop=(dt == DT - 1))
                    # RoPE for q, k; plain copy-out for v
                    for h in range(HPC):
                        for t, dram, scale in (("q", qT_d, SCALE), ("k", kT_d, 1.0)):
                            til = spool.tile([128, SC_W], bf16, tag="til")
                            nc.scalar.activation(til[:], ps[(t, h)][:], Copy,
                                                 scale=scale)
                            rp = rotps.tile([128, SC_W], fp32, tag="rot")
                            nc.tensor.matmul(rp[:], rot_sb[:], til[:],
                                             start=True, stop=True)
                            t1 = tpool.tile([128, SC_W], bf16, tag="t1")
                            nc.vector.tensor_mul(t1[:], til[:],
                                                 cos_sb[:, s0:s0 + SC_W])
                            hat = spool.tile([128, SC_W], bf16, tag="hat")
                            nc.vector.tensor_mul(hat[:], rp[:],
                                                 sin_sb[:, s0:s0 + SC_W])
                            nc.vector.tensor_add(hat[:], hat[:], t1[:])
                            nc.sync.dma_start(
                                dram[h, :, sc * SC_W:(sc + 1) * SC_W], hat[:])
                        vb = spool.tile([128, SC_W], bf16, tag="vb")
                        nc.scalar.copy(vb[:], ps[("v", h)][:])
                        nc.sync.dma_start(
                            vT_d[h, :, sc * SC_W:(sc + 1) * SC_W], vb[:])

            # ================= phase 2: attention + O-proj + RS ===========
            # scoresT route: scoresT[k,q] = kT_tile.T @ qT (born transposed),
            # exp lands directly in the PV staging buffer; column sums on
            # GpSimd; normalization deferred to the output tile.
            with (
                tc.tile_pool(name="qk_sb", bufs=2) as qkpool,
                tc.tile_pool(name="vbf", bufs=2) as vpool,
                tc.tile_pool(name="probsT", bufs=2) as ptpool,
                tc.tile_pool(name="outT", bufs=2) as opool,
                tc.tile_pool(name="small", bufs=4) as smpool,
                tc.tile_pool(name="partial", bufs=3) as papool,
                tc.tile_pool(name="scps", bufs=2, space="PSUM") as scps,
                tc.tile_pool(name="sups", bufs=1, space="PSUM") as sups,
                tc.tile_pool(name="ops", bufs=2, space="PSUM") as ops,
                tc.tile_pool(name="pps", bufs=3, space="PSUM") as pps,
            ):
                for b in range(B):
                    oT = {}
                    for h in range(HPC):
                        qT = qkpool.tile([128, S], bf16, tag="qT")
                        nc.sync.dma_start(qT[:], qT_d[h, :, b * S:(b + 1) * S])
                        kT = qkpool.tile([128, S], bf16, tag="kT")
                        nc.sync.dma_start(kT[:], kT_d[h, :, b * S:(b + 1) * S])
                        vbf = vpool.tile([128, NQT, HD], bf16, tag="v")
                        for kt in range(NQT):
                            nc.sync.dma_start(
                                vbf[:, kt, :],
                                vT_d[h, :, b * S + kt * 128: b * S + (kt + 1) * 128],
                                transpose=True)
                        oT[h] = opool.tile([128, S], bf16, tag="oT", name=f"oT{h}")

                        for qg in range(NQG):
                            kmax = qg * 4 + 3
                            pT = ptpool.tile([128, NQT, 512], bf16, tag="pT")
                            po = ops.tile([128, 512], fp32, tag="po")
                            sums_ps = sups.tile([1, 512], fp32, tag="sps")
                            for kt in range(kmax + 1):
                                qlo = max(0, kt - qg * 4) * 128
                                n = 512 - qlo
                                sp = scps.tile([128, 512], fp32, tag="sc")
                                nc.tensor.matmul(
                                    sp[:, :n],
                                    kT[:, kt * 128:(kt + 1) * 128],
                                    qT[:, qg * 512 + qlo: (qg + 1) * 512],
                                    start=True, stop=True)
                                if kt >= qg * 4:  # diag tile sits at local cols 0:128
                                    nc.vector.tensor_add(
                                        sp[:, 0:128], sp[:, 0:128], mdg_sb[:, kt, :])
                                nc.scalar.activation(
                                    pT[:, kt, qlo:512], sp[:, :n], Exp)
                                if kt >= 1:
                                    klast = kt - 1
                                    ql2 = max(0, klast - qg * 4) * 128
                                    nc.tensor.matmul(
                                        po[:, ql2:512], vbf[:, klast, :],
                                        pT[:, klast, ql2:512],
                                        start=(klast == 0), stop=False)
                                    nc.tensor.matmul(
                                        sums_ps[:, ql2:512], one_sb[:],
                                        pT[:, klast, ql2:512],
                                        start=(klast == 0), stop=False)
                            nc.tensor.matmul(
                                po[:, 384:512], vbf[:, kmax, :],
                                pT[:, kmax, 384:512], start=False, stop=True)
                            nc.tensor.matmul(
                                sums_ps[:, 384:512], one_sb[:],
                                pT[:, kmax, 384:512], start=False, stop=True)
                            # normalization for this q-column group
                            srow = smpool.tile([1, 512], fp32, tag="srow")
                            nc.scalar.copy(srow[:], sums_ps[:])
                            sbc = smpool.tile([128, 512], fp32, tag="sbc")
                            nc.gpsimd.partition_broadcast(sbc[:], srow[:])
                            rbc = smpool.tile([128, 512], fp32, tag="rbc")
                            nc.vector.reciprocal(rbc[:], sbc[:])
                            nc.vector.tensor_mul(
                                oT[h][:, qg * 512:(qg + 1) * 512], po[:], rbc[:])

                    # ---- O-projection for batch b + chunked ReduceScatter ----
                    for st in range(NQT):
                        pp = [pps.tile([128, 512], fp32, tag="pp", name=f"pp{e}") for e in range(4)]
                        for h in range(HPC):
                            for ec in range(4):
                                nc.tensor.matmul(
                                    pp[ec][:],
                                    oT[h][:, st * 128:(st + 1) * 128],
                                    wot_sb[:, h, ec * 512:(ec + 1) * 512],
                                    start=(h == 0), stop=(h == HPC - 1))
                        par = papool.tile([128, DIM], bf16, tag="par")
                        for ec in range(4):
                            eng = nc.scalar if ec % 2 == 0 else nc.vector
                            if ec % 2 == 0:
                                nc.scalar.copy(par[:, ec * 512:(ec + 1) * 512], pp[ec][:])
                            else:
                                nc.vector.tensor_copy(par[:, ec * 512:(ec + 1) * 512], pp[ec][:])
                        nc.sync.dma_start(
                            par_d[b * S + st * 128: b * S + (st + 1) * 128, :], par[:])
                        # ReduceScatter every RS_ROWS rows
                        if (st + 1) % (RS_ROWS // 128) == 0:
                            ch = (b * S + (st + 1) * 128) // RS_ROWS - 1
                            nc.gpsimd.collective_compute(
                                "ReduceScatter", ADD, replica_groups=rg,
                                ins=[par_d[ch * RS_ROWS:(ch + 1) * RS_ROWS, :]],
                                outs=[rs_d[ch]])
                            nc.sync.dma_start(out_d[ch], rs_d[ch])

    nc.compile()
    return nc


def _get_nc(S):
    if S not in _CACHE:
        _CACHE[S] = _build(S)
    return _CACHE[S]


def make_inputs(x, freqs_cis, mask, wq, wk, wv, wo):
    """Host-side sharding / layout prep. Returns in_maps for 8 cores."""
    S = x.shape[1]
    flat_xt = np.ascontiguousarray(np.asarray(x, np.float32).reshape(B * S, DIM).T)
    cos = np.asarray(freqs_cis[..., 0], np.float32)   # [S, HD/2]
    sin = np.asarray(freqs_cis[..., 1], np.float32)
    cos_t = np.ascontiguousarray(np.repeat(cos.T, 2, axis=0))  # [HD, S]
    sin_t = np.ascontiguousarray(np.repeat(sin.T, 2, axis=0))
    m = np.asarray(mask, np.float32)[0, 0]
    nqt = S // 128
    mask_diag = np.ascontiguousarray(
        np.stack([m[i * 128:(i + 1) * 128, i * 128:(i + 1) * 128].T
                  for i in range(nqt)]))
    import ml_dtypes
    bf = ml_dtypes.bfloat16
    flat_xt = flat_xt.astype(bf)
    cos_t = cos_t.astype(bf)
    sin_t = sin_t.astype(bf)
    ident_bf = np.eye(128, dtype=bf)
    P = np.zeros((128, 128), np.float32)
    for j in range(64):
        P[2 * j, 2 * j + 1] = -1.0
        P[2 * j + 1, 2 * j] = 1.0
    rotp = np.ascontiguousarray(P.T)

    in_maps = []
    for c in range(N_CORES):
        r = slice(c * OC, (c + 1) * OC)
        in_maps.append({
            "xt": flat_xt,
            "wqt": np.ascontiguousarray(np.asarray(wq, np.float32)[r, :].T).astype(bf),
            "wkt": np.ascontiguousarray(np.asarray(wk, np.float32)[r, :].T).astype(bf),
            "wvt": np.ascontiguousarray(np.asarray(wv, np.float32)[r, :].T).astype(bf),
            "wot": np.ascontiguousarray(np.asarray(wo, np.float32)[:, r].T).astype(bf),
            "cos_t": cos_t,
            "sin_t": sin_t,
            "mask_diag": mask_diag,
            "ident_bf": ident_bf,
            "rotp": rotp.astype(bf),
            "ones_bf": np.ones((128, 1), dtype=bf),
        })
    return in_maps


def assemble(results, S):
    """Concatenate per-core ReduceScatter shards into the full output."""
    nch = B * S // RS_ROWS
    shards = [np.asarray(results[c]["out"], dtype=np.float32)
                .reshape(nch, RS_ROWS // N_CORES, DIM)
              for c in range(N_CORES)]
    full = np.empty((nch, N_CORES, RS_ROWS // N_CORES, DIM), np.float32)
    for c in range(N_CORES):
        full[:, c] = shards[c]
    return full.reshape(B, S, DIM)


def kernel(x, start_pos, freqs_cis, mask, wq, wk, wv, wo):
    from concourse.bass_utils import run_bass_kernel_spmd
    S = x.shape[1]
    nc = _get_nc(S)
    in_maps = make_inputs(x, freqs_cis, mask, wq, wk, wv, wo)
    res = run_bass_kernel_spmd(nc, in_maps, core_ids=list(range(N_CORES)))
    return assemble(res.results, S)
